# revision 53
# baseline (speedup 1.0000x reference)
"""InfoVAE loss kernel for Trainium2, data-parallel over batch on 8 NeuronCores.

Reference computation (see problem spec):
    recons_loss = mean((recons - x)^2)                    recons/x: [4096, 3, 64, 64]
    mmd  = km(pz,pz) + km(z,z) - 2*km(pz,z)               z/pz:     [4096, 128]
           where km(a,b) = mean_ij exp(-(|a_i-b_j|^2/D)/sigma), sigma = 2*D*z_var
    kld  = mean_n(-0.5 * sum_d(1 + lv - mu^2 - exp(lv)))
    loss = 5*recons_loss + 1.5*(1/N)*kld + 98.5/(N*(N-1))*mmd
    returns (loss, recons_loss, mmd, -kld)

Sharding: each core owns a 512-row block of the batch. The RBF kernel blocks are
computed as block-rows vs the full gathered z/prior_z. Layout prep happens on
the host as part of the sharding step: z/prior_z are shipped pre-transposed in
bf16 ([D, N] for the rhs, [D, rows]/32768 for the block lhsT), along with the
tiny per-row norm rows (-|b_j|^2/65536, bf16) and per-partition bias columns
(-|a_i|^2/65536, f32) computed exactly in float64. That removes the on-device
transpose prologue entirely and cuts per-core HBM traffic to ~50 MiB.

RBF assembly on device: arg_ij = a_i.b_j/32768 - |a_i|^2/65536 - |b_j|^2/65536.
 - a_i.b_j/32768 : PE matmul in bf16 (1 cyc/row vs fp32's 4).
 - -|b_j|^2/65536: a K=1 accumulating bf16 matmul (ones outer-product row term).
 - -|a_i|^2/65536: fp32 per-partition bias of the ACT Exp instruction.
ACT's fused accum_out gives the per-partition running sums for free; the Exp
output is written back in place over its PSUM input (no SBUF scratch).

The MSE stream is the DMA floor (48 MiB/core); its compute is spread so no one
engine gates the stream: the subtract runs on DVE, the square+accumulate
alternates between ACT (activation Square) and GpSimd (scalar_tensor_tensor,
otherwise idle) into two separate accumulator tiles so the engines never
serialize on a shared tile.
"""

import numpy as np

N = 4096
D = 128
NCORES = 8
ROWS = N // NCORES            # 512 rows per core
IMG_F = 3 * 64 * 64           # 12288
P = 128
T_ROW = ROWS // P             # 4 row tiles per core
MSE_CHUNK = 2048
MSE_NCH = IMG_F // MSE_CHUNK  # 6
# MSE stream pieces: full 1-MiB-per-tensor chunks for the bulk, the final
# eight chunks split in half so slot turnover speeds up as the stream ends
# (a full-size GpSimd sub is ~4.5 us and gates the tail otherwise)
MSE_PIECES = []
for _k in range(T_ROW * MSE_NCH):
    _t, _c = divmod(_k, MSE_NCH)
    if _k < T_ROW * MSE_NCH - 8:
        MSE_PIECES.append((_t, _c * MSE_CHUNK, MSE_CHUNK))
    else:
        MSE_PIECES.append((_t, _c * MSE_CHUNK, MSE_CHUNK // 2))
        MSE_PIECES.append((_t, _c * MSE_CHUNK + MSE_CHUNK // 2, MSE_CHUNK // 2))
JG = 1024                     # psum group width for the rbf matmuls
NJG = N // JG                 # 4 j-groups
Z_VAR = 2.0
SIGMA = 2.0 * D * Z_VAR       # 512
INV_2S = 1.0 / (D * SIGMA / 2.0)   # 1/32768 (exact power of two)
INV_S = 1.0 / (D * SIGMA)          # 1/65536

NMSE = len(MSE_PIECES)            # 28 accum columns
NMMD = 3 * T_ROW * NJG            # 48 accum columns

_CACHE = {}


def _build():
    import concourse.bass as bass
    import concourse.tile as tile
    from concourse import bacc, mybir

    f32 = mybir.dt.float32
    bf16 = mybir.dt.bfloat16
    AF = mybir.ActivationFunctionType
    ALU = mybir.AluOpType
    AX = mybir.AxisListType

    nc = bacc.Bacc("TRN2", target_bir_lowering=False, debug=False,
                   num_devices=NCORES)

    r_blk = nc.dram_tensor("r_blk", [ROWS, IMG_F], f32, kind="ExternalInput").ap()
    x_blk = nc.dram_tensor("x_blk", [ROWS, IMG_F], f32, kind="ExternalInput").ap()
    zT_in = nc.dram_tensor("zT", [D, N], bf16, kind="ExternalInput").ap()
    pzT_in = nc.dram_tensor("pzT", [D, N], bf16, kind="ExternalInput").ap()
    zbT_in = nc.dram_tensor("zbTs", [D, ROWS], bf16, kind="ExternalInput").ap()
    pzbT_in = nc.dram_tensor("pzbTs", [D, ROWS], bf16, kind="ExternalInput").ap()
    nnz_in = nc.dram_tensor("nn_z", [1, N], bf16, kind="ExternalInput").ap()
    nnpz_in = nc.dram_tensor("nn_pz", [1, N], bf16, kind="ExternalInput").ap()
    bz_in = nc.dram_tensor("bias_z", [P, T_ROW], f32, kind="ExternalInput").ap()
    bpz_in = nc.dram_tensor("bias_pz", [P, T_ROW], f32, kind="ExternalInput").ap()
    mu_blk = nc.dram_tensor("mu_blk", [ROWS, D], f32, kind="ExternalInput").ap()
    lv_blk = nc.dram_tensor("lv_blk", [ROWS, D], f32, kind="ExternalInput").ap()

    mse_out = nc.dram_tensor("mse_acc", [P, NMSE], f32, kind="ExternalOutput").ap()
    mse_a_out = nc.dram_tensor("mse_acc_a", [P, NMSE], f32, kind="ExternalOutput").ap()
    mmd_out = nc.dram_tensor("mmd_acc", [P, NMMD], f32, kind="ExternalOutput").ap()
    kld_out = nc.dram_tensor("kld_acc", [P, 4], f32, kind="ExternalOutput").ap()

    with tile.TileContext(nc) as tc:
        with (
            tc.tile_pool(name="consts", bufs=1) as consts,
            tc.tile_pool(name="nat", bufs=1) as nat,
            tc.tile_pool(name="stream", bufs=8) as stream,
            tc.tile_pool(name="dpool", bufs=4) as dpool,
            tc.tile_pool(name="scratch", bufs=2) as scratch,
            tc.tile_pool(name="acc", bufs=1) as accp,
            tc.tile_pool(name="psmm", bufs=4, space="PSUM") as psmm,
        ):
            rv = r_blk.rearrange("(t p) f -> p t f", p=P)
            xv = x_blk.rearrange("(t p) f -> p t f", p=P)

            # accumulators; mse_cols is DVE-written (bulk), mse_cols_a is
            # ACT-written (tail pieces) — separate tiles so the engines never
            # serialize on shared state; unwritten columns are zeroed since
            # combine() sums both tiles fully
            mse_cols = accp.tile([P, NMSE], f32)
            mse_cols_a = accp.tile([P, NMSE], f32)
            mmd_cols = accp.tile([P, NMMD], f32)
            kld_cols = accp.tile([P, 4], f32)
            nc.vector.memset(mse_cols[:], 0.0)
            nc.vector.memset(mse_cols_a[:], 0.0)
            nc.vector.memset(kld_cols[:, 3:4], 0.0)

            def emit_mse(k, tail=False):
                t, lo, w = MSE_PIECES[k]
                rt = stream.tile([P, w], f32, tag="rt")
                xt = stream.tile([P, w], f32, tag="xt")
                nc.sync.dma_start(out=rt[:], in_=rv[:, t, lo:lo + w])
                nc.sync.dma_start(out=xt[:], in_=xv[:, t, lo:lo + w])
                # subs alternate DVE / GpSimd (a 50/50 split measures best;
                # skewing either way makes that engine the end-of-stream
                # straggler). Bulk pieces: d = r - x in place over rt, then a
                # DVE f32 scalar_tensor_tensor square overwriting xt — ACT is
                # deliberately NOT in this chain, its in-order queue is full
                # of MMD exps that wait on matmuls and would stall the
                # stream-slot release. Tail pieces (emitted after the last
                # exp): d goes to a small bf16 dt tile so rt/xt release at
                # the sub, and the square runs on the by-then-idle ACT; the
                # dt slots absorb any squares still queued behind exps.
                if tail:
                    # tail squares alternate ACT / DVE so the post-exp drain
                    # runs two pieces at a time instead of ACT-serial
                    if k % 2 == 0:
                        dt = dpool.tile([P, w], bf16, tag="dt")
                        nc.vector.tensor_sub(dt[:], rt[:], xt[:])
                        nc.scalar.activation(out=dt[:], in_=dt[:],
                                             func=AF.Square,
                                             accum_out=mse_cols_a[:, k:k + 1])
                    else:
                        dtf = dpool.tile([P, w], f32, tag="dtf")
                        nc.gpsimd.tensor_sub(dtf[:], rt[:], xt[:])
                        nc.vector.scalar_tensor_tensor(
                            out=dtf[:], in0=dtf[:], scalar=1.0, in1=dtf[:],
                            op0=ALU.mult, op1=ALU.mult,
                            accum_out=mse_cols[:, k:k + 1])
                else:
                    if k % 2 == 0:
                        nc.vector.tensor_sub(rt[:], rt[:], xt[:])
                    else:
                        nc.gpsimd.tensor_sub(rt[:], rt[:], xt[:])
                    nc.vector.scalar_tensor_tensor(
                        out=xt[:], in0=rt[:], scalar=1.0, in1=rt[:],
                        op0=ALU.mult, op1=ALU.mult,
                        accum_out=mse_cols[:, k:k + 1])

            # ---- constants / small setup (all layouts host-prepared; these
            # small transfers go first so the PE pipeline starts early) ----
            ones_row = consts.tile([1, P], bf16)
            nc.vector.memset(ones_row[:], 1.0)

            zT = consts.tile([P, N], bf16)
            pzT = consts.tile([P, N], bf16)
            zbTs = consts.tile([P, ROWS], bf16)
            pbTs = consts.tile([P, ROWS], bf16)
            nn_z = consts.tile([1, N], bf16)
            nn_pz = consts.tile([1, N], bf16)
            bias_z = consts.tile([P, T_ROW], f32)
            bias_pz = consts.tile([P, T_ROW], f32)
            # two big MSE pieces first so the DMA pipe is deep from the very
            # first issue (the small prologue loads alone leave the SDMA
            # engines underfed), then the prologue, then two more pieces
            emit_mse(0)
            emit_mse(1)
            nc.sync.dma_start(out=zT[:], in_=zT_in)
            nc.sync.dma_start(out=pzT[:], in_=pzT_in)
            nc.sync.dma_start(out=zbTs[:], in_=zbT_in)
            nc.sync.dma_start(out=pbTs[:], in_=pzbT_in)
            emit_mse(2)
            nc.sync.dma_start(out=nn_z[:], in_=nnz_in)
            nc.sync.dma_start(out=nn_pz[:], in_=nnpz_in)
            nc.sync.dma_start(out=bias_z[:], in_=bz_in)
            nc.sync.dma_start(out=bias_pz[:], in_=bpz_in)
            emit_mse(3)

            # ---- KLD block terms ----
            mu_t = nat.tile([P, T_ROW, D], f32)
            lv_t = nat.tile([P, T_ROW, D], f32)
            nc.sync.dma_start(out=mu_t[:], in_=mu_blk.rearrange("(t p) d -> p t d", p=P))
            nc.sync.dma_start(out=lv_t[:], in_=lv_blk.rearrange("(t p) d -> p t d", p=P))
            ksc = scratch.tile([P, T_ROW, D], f32, tag="ksc")
            nc.vector.tensor_reduce(kld_cols[:, 0:1], lv_t[:], axis=AX.XY,
                                    op=ALU.add)
            nc.scalar.activation(out=ksc[:], in_=mu_t[:], func=AF.Square,
                                 accum_out=kld_cols[:, 1:2])
            ksc2 = scratch.tile([P, T_ROW, D], f32, tag="ksc")
            nc.scalar.activation(out=ksc2[:], in_=lv_t[:], func=AF.Exp,
                                 accum_out=kld_cols[:, 2:3])
            # kld is complete here — write it back now, off the congested
            # end-of-kernel queue position
            nc.sync.dma_start(out=kld_out, in_=kld_cols[:])

            # ---- interleaved main loops: MMD rbf blocks + MSE stream ----
            pairs = [(pbTs, pzT, nn_pz, bias_pz),   # k(pz, pz)
                     (zbTs, zT, nn_z, bias_z),      # k(z, z)
                     (pbTs, zT, nn_z, bias_pz)]     # k(pz, z)

            def emit_mmd_mms(k):
                pi, rem = divmod(k, T_ROW * NJG)
                t, jg = divmod(rem, NJG)
                aTs, bT, nn_b, bias_a = pairs[pi]
                ps = psmm.tile([P, JG], f32, tag="mm")
                for jc in range(JG // 512):
                    j = jg * (JG // 512) + jc
                    nc.tensor.matmul(ps[:, jc * 512:(jc + 1) * 512],
                                     lhsT=aTs[:, t * P:(t + 1) * P],
                                     rhs=bT[:, j * 512:(j + 1) * 512],
                                     start=True, stop=False)
                    nc.tensor.matmul(ps[:, jc * 512:(jc + 1) * 512],
                                     lhsT=ones_row[:], rhs=nn_b[0:1, j * 512:(j + 1) * 512],
                                     start=False, stop=True)
                return ps, bias_a, t

            def emit_mmd_exp(k, ps, bias_a, t):
                # exp in place over the psum tile; accum_out is the partial sum
                nc.scalar.activation(out=ps[:], in_=ps[:], func=AF.Exp,
                                     bias=bias_a[:, t:t + 1], scale=1.0,
                                     accum_out=mmd_cols[:, k:k + 1])

            # Emit matmuls two groups at a time: 8 back-to-back matmuls are
            # ~4.9 us of continuous PE work, enough to cross the ~3.4 us HAM
            # activity window so the PE clock lifts to 2.4 GHz (single groups
            # of ~2.4 us never warm it). The MSE stream is front-loaded: one
            # piece per group until fully issued, so its DMA + consumption
            # finish well before the MMD exp tail.
            n_bulk = NMSE - 8
            for kk in range(0, NMMD, 2):
                g0 = emit_mmd_mms(kk)
                g1 = emit_mmd_mms(kk + 1)
                emit_mmd_exp(kk, *g0)
                emit_mmd_exp(kk + 1, *g1)
                if kk + 4 < n_bulk:
                    emit_mse(kk + 4)
                if kk + 5 < n_bulk:
                    emit_mse(kk + 5)

            # half-size tail pieces, emitted after every exp so their ACT
            # squares queue behind nothing
            for k in range(n_bulk, NMSE):
                emit_mse(k, tail=True)

            # ---- write partials out (the MSE accumulators finish last) ----
            nc.sync.dma_start(out=mmd_out, in_=mmd_cols[:])
            nc.sync.dma_start(out=mse_out, in_=mse_cols[:])
            nc.sync.dma_start(out=mse_a_out, in_=mse_cols_a[:])

    nc.compile()
    return nc


def get_nc():
    if "nc" not in _CACHE:
        _CACHE["nc"] = _build()
    return _CACHE["nc"]


def make_in_maps(recons, x, z, mu, log_var, prior_z):
    import ml_dtypes
    bf = ml_dtypes.bfloat16

    r2 = np.ascontiguousarray(recons, dtype=np.float32).reshape(N, IMG_F)
    x2 = np.ascontiguousarray(x, dtype=np.float32).reshape(N, IMG_F)
    z64 = np.asarray(z, np.float64)
    pz64 = np.asarray(prior_z, np.float64)
    mu = np.ascontiguousarray(mu, dtype=np.float32)
    lv = np.ascontiguousarray(log_var, dtype=np.float32)

    # host-side layout prep (the "gather + shard" step): bf16 transposed
    # copies, exact norm rows and bias columns
    zbf = z64.astype(bf)
    pzbf = pz64.astype(bf)
    zT = np.ascontiguousarray(zbf.T)                              # [D, N]
    pzT = np.ascontiguousarray(pzbf.T)
    zbs = np.ascontiguousarray((z64 * INV_2S).astype(bf).T)       # [D, N] /2^15
    pzbs = np.ascontiguousarray((pz64 * INV_2S).astype(bf).T)
    # norms of the bf16-rounded values (matches the matmul operands)
    nn_z = (-np.sum(zbf.astype(np.float64) ** 2, axis=1) * INV_S).astype(bf)[None, :]
    nn_pz = (-np.sum(pzbf.astype(np.float64) ** 2, axis=1) * INV_S).astype(bf)[None, :]
    bias_z = (-np.sum(zbf.astype(np.float64) ** 2, axis=1) * INV_S).astype(np.float32)
    bias_pz = (-np.sum(pzbf.astype(np.float64) ** 2, axis=1) * INV_S).astype(np.float32)
    # bias laid out [P, T_ROW] per core block: row i of block -> (t, p) = divmod(i, P)
    bias_z = bias_z.reshape(NCORES, T_ROW, P).transpose(0, 2, 1)   # [c, P, T_ROW]
    bias_pz = bias_pz.reshape(NCORES, T_ROW, P).transpose(0, 2, 1)

    maps = []
    for c in range(NCORES):
        s = slice(c * ROWS, (c + 1) * ROWS)
        maps.append({
            "r_blk": r2[s], "x_blk": x2[s],
            "zT": zT, "pzT": pzT,
            "zbTs": np.ascontiguousarray(zbs[:, s]),
            "pzbTs": np.ascontiguousarray(pzbs[:, s]),
            "nn_z": nn_z, "nn_pz": nn_pz,
            "bias_z": np.ascontiguousarray(bias_z[c]),
            "bias_pz": np.ascontiguousarray(bias_pz[c]),
            "mu_blk": mu[s], "lv_blk": lv[s],
        })
    return maps


def combine(results):
    mse_sum = 0.0
    s_pp = s_zz = s_pz = 0.0
    kld_total = 0.0
    per_pair = T_ROW * NJG
    for res in results:
        mse_sum += np.float64(res["mse_acc"]).sum()
        mse_sum += np.float64(res["mse_acc_a"]).sum()
        m = np.float64(res["mmd_acc"])
        s_pp += m[:, 0:per_pair].sum()
        s_zz += m[:, per_pair:2 * per_pair].sum()
        s_pz += m[:, 2 * per_pair:3 * per_pair].sum()
        k = np.float64(res["kld_acc"])
        kld_total += ROWS * D + k[:, 0].sum() - k[:, 1].sum() - k[:, 2].sum()

    recons_loss = mse_sum / (N * IMG_F)
    mmd = (s_pp + s_zz - 2.0 * s_pz) / (float(N) * float(N))
    kld = -0.5 * kld_total / N
    beta, alpha, reg_w = 5.0, -0.5, 100.0
    loss = (beta * recons_loss
            + (1.0 - alpha) * (1.0 / N) * kld
            + (alpha + reg_w - 1.0) / (float(N) * (N - 1)) * mmd)
    return (np.float32(loss), np.float32(recons_loss),
            np.float32(mmd), np.float32(-kld))


def run(recons, x, z, mu, log_var, prior_z, trace=False):
    from concourse.bass_utils import run_bass_kernel_spmd
    nc = get_nc()
    in_maps = make_in_maps(recons, x, z, mu, log_var, prior_z)
    res = run_bass_kernel_spmd(nc, in_maps, list(range(NCORES)), trace=trace)
    return res


def kernel(recons, x, z, mu, log_var, prior_z):
    res = run(recons, x, z, mu, log_var, prior_z)
    return combine(res.results)


# revision 54
# speedup vs baseline: 1.0171x; 1.0171x over previous
"""InfoVAE loss kernel for Trainium2, data-parallel over batch on 8 NeuronCores.

Reference computation (see problem spec):
    recons_loss = mean((recons - x)^2)                    recons/x: [4096, 3, 64, 64]
    mmd  = km(pz,pz) + km(z,z) - 2*km(pz,z)               z/pz:     [4096, 128]
           where km(a,b) = mean_ij exp(-(|a_i-b_j|^2/D)/sigma), sigma = 2*D*z_var
    kld  = mean_n(-0.5 * sum_d(1 + lv - mu^2 - exp(lv)))
    loss = 5*recons_loss + 1.5*(1/N)*kld + 98.5/(N*(N-1))*mmd
    returns (loss, recons_loss, mmd, -kld)

Sharding: each core owns a 512-row block of the batch. The RBF kernel blocks are
computed as block-rows vs the full gathered z/prior_z. Layout prep happens on
the host as part of the sharding step: z/prior_z are shipped pre-transposed in
bf16 ([D, N] for the rhs, [D, rows]/32768 for the block lhsT), along with the
tiny per-row norm rows (-|b_j|^2/65536, bf16) and per-partition bias columns
(-|a_i|^2/65536, f32) computed exactly in float64. That removes the on-device
transpose prologue entirely and cuts per-core HBM traffic to ~50 MiB.

RBF assembly on device: arg_ij = a_i.b_j/32768 - |a_i|^2/65536 - |b_j|^2/65536.
 - a_i.b_j/32768 : PE matmul in bf16 (1 cyc/row vs fp32's 4).
 - -|b_j|^2/65536: a K=1 accumulating bf16 matmul (ones outer-product row term).
 - -|a_i|^2/65536: fp32 per-partition bias of the ACT Exp instruction.
ACT's fused accum_out gives the per-partition running sums for free; the Exp
output is written back in place over its PSUM input (no SBUF scratch).

The MSE stream is the DMA floor (48 MiB/core); its compute is spread so no one
engine gates the stream: the subtract runs on DVE, the square+accumulate
alternates between ACT (activation Square) and GpSimd (scalar_tensor_tensor,
otherwise idle) into two separate accumulator tiles so the engines never
serialize on a shared tile.
"""

import numpy as np

N = 4096
D = 128
NCORES = 8
ROWS = N // NCORES            # 512 rows per core
IMG_F = 3 * 64 * 64           # 12288
P = 128
T_ROW = ROWS // P             # 4 row tiles per core
MSE_CHUNK = 2048
MSE_NCH = IMG_F // MSE_CHUNK  # 6
# MSE stream pieces: full 1-MiB-per-tensor chunks for the bulk, the final
# eight chunks split in half so slot turnover speeds up as the stream ends
# (a full-size GpSimd sub is ~4.5 us and gates the tail otherwise)
MSE_PIECES = []
for _k in range(T_ROW * MSE_NCH):
    _t, _c = divmod(_k, MSE_NCH)
    if _k < T_ROW * MSE_NCH - 8:
        MSE_PIECES.append((_t, _c * MSE_CHUNK, MSE_CHUNK))
    else:
        MSE_PIECES.append((_t, _c * MSE_CHUNK, MSE_CHUNK // 2))
        MSE_PIECES.append((_t, _c * MSE_CHUNK + MSE_CHUNK // 2, MSE_CHUNK // 2))
JG = 1024                     # psum group width for the rbf matmuls
NJG = N // JG                 # 4 j-groups
Z_VAR = 2.0
SIGMA = 2.0 * D * Z_VAR       # 512
INV_2S = 1.0 / (D * SIGMA / 2.0)   # 1/32768 (exact power of two)
INV_S = 1.0 / (D * SIGMA)          # 1/65536

NMSE = len(MSE_PIECES)            # 28 accum columns
NMMD = 3 * T_ROW * NJG            # 48 accum columns

_CACHE = {}


def _build():
    import concourse.bass as bass
    import concourse.tile as tile
    from concourse import bacc, mybir

    f32 = mybir.dt.float32
    bf16 = mybir.dt.bfloat16
    AF = mybir.ActivationFunctionType
    ALU = mybir.AluOpType
    AX = mybir.AxisListType

    nc = bacc.Bacc("TRN2", target_bir_lowering=False, debug=False,
                   num_devices=NCORES)

    r_blk = nc.dram_tensor("r_blk", [ROWS, IMG_F], f32, kind="ExternalInput").ap()
    x_blk = nc.dram_tensor("x_blk", [ROWS, IMG_F], f32, kind="ExternalInput").ap()
    zT_in = nc.dram_tensor("zT", [D, N], bf16, kind="ExternalInput").ap()
    pzT_in = nc.dram_tensor("pzT", [D, N], bf16, kind="ExternalInput").ap()
    zbT_in = nc.dram_tensor("zbTs", [D, ROWS], bf16, kind="ExternalInput").ap()
    pzbT_in = nc.dram_tensor("pzbTs", [D, ROWS], bf16, kind="ExternalInput").ap()
    nnz_in = nc.dram_tensor("nn_z", [1, N], bf16, kind="ExternalInput").ap()
    nnpz_in = nc.dram_tensor("nn_pz", [1, N], bf16, kind="ExternalInput").ap()
    bz_in = nc.dram_tensor("bias_z", [P, T_ROW], f32, kind="ExternalInput").ap()
    bpz_in = nc.dram_tensor("bias_pz", [P, T_ROW], f32, kind="ExternalInput").ap()
    mu_blk = nc.dram_tensor("mu_blk", [ROWS, D], f32, kind="ExternalInput").ap()
    lv_blk = nc.dram_tensor("lv_blk", [ROWS, D], f32, kind="ExternalInput").ap()

    mse_out = nc.dram_tensor("mse_acc", [P, NMSE], f32, kind="ExternalOutput").ap()
    mse_a_out = nc.dram_tensor("mse_acc_a", [P, NMSE], f32, kind="ExternalOutput").ap()
    mmd_out = nc.dram_tensor("mmd_acc", [P, NMMD], f32, kind="ExternalOutput").ap()
    kld_out = nc.dram_tensor("kld_acc", [P, 4], f32, kind="ExternalOutput").ap()

    with tile.TileContext(nc) as tc:
        with (
            tc.tile_pool(name="consts", bufs=1) as consts,
            tc.tile_pool(name="nat", bufs=1) as nat,
            tc.tile_pool(name="stream", bufs=8) as stream,
            tc.tile_pool(name="dpool", bufs=4) as dpool,
            tc.tile_pool(name="scratch", bufs=2) as scratch,
            tc.tile_pool(name="acc", bufs=1) as accp,
            tc.tile_pool(name="psmm", bufs=4, space="PSUM") as psmm,
        ):
            rv = r_blk.rearrange("(t p) f -> p t f", p=P)
            xv = x_blk.rearrange("(t p) f -> p t f", p=P)

            # accumulators; mse_cols is DVE-written (bulk), mse_cols_a is
            # ACT-written (tail pieces) — separate tiles so the engines never
            # serialize on shared state; unwritten columns are zeroed since
            # combine() sums both tiles fully
            mse_cols = accp.tile([P, NMSE], f32)
            mse_cols_a = accp.tile([P, NMSE], f32)
            mmd_cols = accp.tile([P, NMMD], f32)
            kld_cols = accp.tile([P, 4], f32)
            nc.vector.memset(mse_cols[:], 0.0)
            nc.vector.memset(mse_cols_a[:], 0.0)
            nc.vector.memset(kld_cols[:, 3:4], 0.0)

            def emit_mse(k, tail=False):
                t, lo, w = MSE_PIECES[k]
                rt = stream.tile([P, w], f32, tag="rt")
                xt = stream.tile([P, w], f32, tag="xt")
                nc.sync.dma_start(out=rt[:], in_=rv[:, t, lo:lo + w])
                nc.sync.dma_start(out=xt[:], in_=xv[:, t, lo:lo + w])
                # subs alternate DVE / GpSimd (a 50/50 split measures best;
                # skewing either way makes that engine the end-of-stream
                # straggler). Bulk pieces: d = r - x in place over rt, then a
                # DVE f32 scalar_tensor_tensor square overwriting xt — ACT is
                # deliberately NOT in this chain, its in-order queue is full
                # of MMD exps that wait on matmuls and would stall the
                # stream-slot release. Tail pieces (emitted after the last
                # exp): d goes to a small bf16 dt tile so rt/xt release at
                # the sub, and the square runs on the by-then-idle ACT; the
                # dt slots absorb any squares still queued behind exps.
                if tail:
                    dt = dpool.tile([P, w], bf16, tag="dt")
                    if k % 2 == 0:
                        nc.vector.tensor_sub(dt[:], rt[:], xt[:])
                    else:
                        nc.gpsimd.tensor_sub(dt[:], rt[:], xt[:])
                    nc.scalar.activation(out=dt[:], in_=dt[:], func=AF.Square,
                                         accum_out=mse_cols_a[:, k:k + 1])
                else:
                    if k % 2 == 0:
                        nc.vector.tensor_sub(rt[:], rt[:], xt[:])
                    else:
                        nc.gpsimd.tensor_sub(rt[:], rt[:], xt[:])
                    nc.vector.scalar_tensor_tensor(
                        out=xt[:], in0=rt[:], scalar=1.0, in1=rt[:],
                        op0=ALU.mult, op1=ALU.mult,
                        accum_out=mse_cols[:, k:k + 1])

            # ---- constants / small setup (all layouts host-prepared; these
            # small transfers go first so the PE pipeline starts early) ----
            ones_row = consts.tile([1, P], bf16)
            nc.vector.memset(ones_row[:], 1.0)

            zT = consts.tile([P, N], bf16)
            pzT = consts.tile([P, N], bf16)
            zbTs = consts.tile([P, ROWS], bf16)
            pbTs = consts.tile([P, ROWS], bf16)
            nn_z = consts.tile([1, N], bf16)
            nn_pz = consts.tile([1, N], bf16)
            bias_z = consts.tile([P, T_ROW], f32)
            bias_pz = consts.tile([P, T_ROW], f32)
            # two big MSE pieces first so the DMA pipe is deep from the very
            # first issue (the small prologue loads alone leave the SDMA
            # engines underfed), then the prologue, then two more pieces
            emit_mse(0)
            emit_mse(1)
            nc.sync.dma_start(out=zT[:], in_=zT_in)
            nc.sync.dma_start(out=pzT[:], in_=pzT_in)
            nc.sync.dma_start(out=zbTs[:], in_=zbT_in)
            nc.sync.dma_start(out=pbTs[:], in_=pzbT_in)
            emit_mse(2)
            nc.sync.dma_start(out=nn_z[:], in_=nnz_in)
            nc.sync.dma_start(out=nn_pz[:], in_=nnpz_in)
            nc.sync.dma_start(out=bias_z[:], in_=bz_in)
            nc.sync.dma_start(out=bias_pz[:], in_=bpz_in)
            emit_mse(3)

            # ---- KLD block terms ----
            mu_t = nat.tile([P, T_ROW, D], f32)
            lv_t = nat.tile([P, T_ROW, D], f32)
            nc.sync.dma_start(out=mu_t[:], in_=mu_blk.rearrange("(t p) d -> p t d", p=P))
            nc.sync.dma_start(out=lv_t[:], in_=lv_blk.rearrange("(t p) d -> p t d", p=P))
            ksc = scratch.tile([P, T_ROW, D], f32, tag="ksc")
            nc.vector.tensor_reduce(kld_cols[:, 0:1], lv_t[:], axis=AX.XY,
                                    op=ALU.add)
            nc.scalar.activation(out=ksc[:], in_=mu_t[:], func=AF.Square,
                                 accum_out=kld_cols[:, 1:2])
            ksc2 = scratch.tile([P, T_ROW, D], f32, tag="ksc")
            nc.scalar.activation(out=ksc2[:], in_=lv_t[:], func=AF.Exp,
                                 accum_out=kld_cols[:, 2:3])
            # kld is complete here — write it back now, off the congested
            # end-of-kernel queue position
            nc.sync.dma_start(out=kld_out, in_=kld_cols[:])

            # ---- interleaved main loops: MMD rbf blocks + MSE stream ----
            pairs = [(pbTs, pzT, nn_pz, bias_pz),   # k(pz, pz)
                     (zbTs, zT, nn_z, bias_z),      # k(z, z)
                     (pbTs, zT, nn_z, bias_pz)]     # k(pz, z)

            def emit_mmd_mms(k):
                pi, rem = divmod(k, T_ROW * NJG)
                t, jg = divmod(rem, NJG)
                aTs, bT, nn_b, bias_a = pairs[pi]
                ps = psmm.tile([P, JG], f32, tag="mm")
                for jc in range(JG // 512):
                    j = jg * (JG // 512) + jc
                    nc.tensor.matmul(ps[:, jc * 512:(jc + 1) * 512],
                                     lhsT=aTs[:, t * P:(t + 1) * P],
                                     rhs=bT[:, j * 512:(j + 1) * 512],
                                     start=True, stop=False)
                    nc.tensor.matmul(ps[:, jc * 512:(jc + 1) * 512],
                                     lhsT=ones_row[:], rhs=nn_b[0:1, j * 512:(j + 1) * 512],
                                     start=False, stop=True)
                return ps, bias_a, t

            def emit_mmd_exp(k, ps, bias_a, t):
                # exp in place over the psum tile; accum_out is the partial sum
                nc.scalar.activation(out=ps[:], in_=ps[:], func=AF.Exp,
                                     bias=bias_a[:, t:t + 1], scale=1.0,
                                     accum_out=mmd_cols[:, k:k + 1])

            # Emit matmuls two groups at a time: 8 back-to-back matmuls are
            # ~4.9 us of continuous PE work, enough to cross the ~3.4 us HAM
            # activity window so the PE clock lifts to 2.4 GHz (single groups
            # of ~2.4 us never warm it). The MSE stream is front-loaded: one
            # piece per group until fully issued, so its DMA + consumption
            # finish well before the MMD exp tail.
            n_bulk = NMSE - 8
            for kk in range(0, NMMD, 2):
                g0 = emit_mmd_mms(kk)
                g1 = emit_mmd_mms(kk + 1)
                emit_mmd_exp(kk, *g0)
                emit_mmd_exp(kk + 1, *g1)
                if kk + 4 < n_bulk:
                    emit_mse(kk + 4)
                if kk + 5 < n_bulk:
                    emit_mse(kk + 5)

            # half-size tail pieces, emitted after every exp so their ACT
            # squares queue behind nothing
            for k in range(n_bulk, NMSE):
                emit_mse(k, tail=True)

            # ---- write partials out (the MSE accumulators finish last) ----
            nc.sync.dma_start(out=mmd_out, in_=mmd_cols[:])
            nc.sync.dma_start(out=mse_out, in_=mse_cols[:])
            nc.sync.dma_start(out=mse_a_out, in_=mse_cols_a[:])

    nc.compile()
    return nc


def get_nc():
    if "nc" not in _CACHE:
        _CACHE["nc"] = _build()
    return _CACHE["nc"]


def make_in_maps(recons, x, z, mu, log_var, prior_z):
    import ml_dtypes
    bf = ml_dtypes.bfloat16

    r2 = np.ascontiguousarray(recons, dtype=np.float32).reshape(N, IMG_F)
    x2 = np.ascontiguousarray(x, dtype=np.float32).reshape(N, IMG_F)
    z64 = np.asarray(z, np.float64)
    pz64 = np.asarray(prior_z, np.float64)
    mu = np.ascontiguousarray(mu, dtype=np.float32)
    lv = np.ascontiguousarray(log_var, dtype=np.float32)

    # host-side layout prep (the "gather + shard" step): bf16 transposed
    # copies, exact norm rows and bias columns
    zbf = z64.astype(bf)
    pzbf = pz64.astype(bf)
    zT = np.ascontiguousarray(zbf.T)                              # [D, N]
    pzT = np.ascontiguousarray(pzbf.T)
    zbs = np.ascontiguousarray((z64 * INV_2S).astype(bf).T)       # [D, N] /2^15
    pzbs = np.ascontiguousarray((pz64 * INV_2S).astype(bf).T)
    # norms of the bf16-rounded values (matches the matmul operands)
    nn_z = (-np.sum(zbf.astype(np.float64) ** 2, axis=1) * INV_S).astype(bf)[None, :]
    nn_pz = (-np.sum(pzbf.astype(np.float64) ** 2, axis=1) * INV_S).astype(bf)[None, :]
    bias_z = (-np.sum(zbf.astype(np.float64) ** 2, axis=1) * INV_S).astype(np.float32)
    bias_pz = (-np.sum(pzbf.astype(np.float64) ** 2, axis=1) * INV_S).astype(np.float32)
    # bias laid out [P, T_ROW] per core block: row i of block -> (t, p) = divmod(i, P)
    bias_z = bias_z.reshape(NCORES, T_ROW, P).transpose(0, 2, 1)   # [c, P, T_ROW]
    bias_pz = bias_pz.reshape(NCORES, T_ROW, P).transpose(0, 2, 1)

    maps = []
    for c in range(NCORES):
        s = slice(c * ROWS, (c + 1) * ROWS)
        maps.append({
            "r_blk": r2[s], "x_blk": x2[s],
            "zT": zT, "pzT": pzT,
            "zbTs": np.ascontiguousarray(zbs[:, s]),
            "pzbTs": np.ascontiguousarray(pzbs[:, s]),
            "nn_z": nn_z, "nn_pz": nn_pz,
            "bias_z": np.ascontiguousarray(bias_z[c]),
            "bias_pz": np.ascontiguousarray(bias_pz[c]),
            "mu_blk": mu[s], "lv_blk": lv[s],
        })
    return maps


def combine(results):
    mse_sum = 0.0
    s_pp = s_zz = s_pz = 0.0
    kld_total = 0.0
    per_pair = T_ROW * NJG
    for res in results:
        mse_sum += np.float64(res["mse_acc"]).sum()
        mse_sum += np.float64(res["mse_acc_a"]).sum()
        m = np.float64(res["mmd_acc"])
        s_pp += m[:, 0:per_pair].sum()
        s_zz += m[:, per_pair:2 * per_pair].sum()
        s_pz += m[:, 2 * per_pair:3 * per_pair].sum()
        k = np.float64(res["kld_acc"])
        kld_total += ROWS * D + k[:, 0].sum() - k[:, 1].sum() - k[:, 2].sum()

    recons_loss = mse_sum / (N * IMG_F)
    mmd = (s_pp + s_zz - 2.0 * s_pz) / (float(N) * float(N))
    kld = -0.5 * kld_total / N
    beta, alpha, reg_w = 5.0, -0.5, 100.0
    loss = (beta * recons_loss
            + (1.0 - alpha) * (1.0 / N) * kld
            + (alpha + reg_w - 1.0) / (float(N) * (N - 1)) * mmd)
    return (np.float32(loss), np.float32(recons_loss),
            np.float32(mmd), np.float32(-kld))


def run(recons, x, z, mu, log_var, prior_z, trace=False):
    from concourse.bass_utils import run_bass_kernel_spmd
    nc = get_nc()
    in_maps = make_in_maps(recons, x, z, mu, log_var, prior_z)
    res = run_bass_kernel_spmd(nc, in_maps, list(range(NCORES)), trace=trace)
    return res


def kernel(recons, x, z, mu, log_var, prior_z):
    res = run(recons, x, z, mu, log_var, prior_z)
    return combine(res.results)
